# revision 45
# baseline (speedup 1.0000x reference)
"""Two-layer GCN (PyG GCNConv style) on 8 Trainium2 NeuronCores — v2.

Math (reference):
    src,dst += self-loops; deg = indeg(dst)+1 ; dinv = deg^-1/2
    norm_e  = dinv[src]*dinv[dst]
    h  = relu( scatter(norm * (x@W1)[src] -> dst) + b1 )
    out =      scatter(norm * (h@W2)[src] -> dst) + b2

Factorization: with P' = dinv (.) (x@W1) = (dinv (.) x)@W1,
    h' = dinv (.) h = relu( dinv^2 * (agg + P'[d] + sqrt(deg)*b1-term) )
    out[d] = dinv[d] * ( (agg2 + h'[d]) @ W2 ) + b2
so scatters are pure segment-sums of gathered rows done as one-hot
matmuls on the PE; epilogues ride the ACT engine (relu/copy with
per-partition scale); self-loop and bias terms are extra PE matmuls.

v2 changes vs v1:
  - AllGather split into 4 tile-quarters (one per gather bank) so the
    collective pipelines with the gather stream; layer-2's AllGather
    overlaps layer-1 aggregation.
  - Feature tables are stored core-major per quarter; gather banks ==
    quarter tables.
  - Per (group,bank,tile) capacities aligned to 32 (not 128-per-tile),
    chunks may span tiles (dual dstloc columns); ~6% padding vs ~25%.
  - GT=8 dst tiles per group: half the dma_gather calls.
  - Epilogues on ACT (scale=dinv^2 / dinv), freeing the DVE for the
    one-hot builds; self-loop + bias folded into PSUM accumulation.
"""

import numpy as np
import ml_dtypes

N = 100000
NCORES = 8
M = N // NCORES          # 12500 nodes per core
IND, HID, OUTD = 256, 128, 237
P = 128
NT = (M + P - 1) // P    # 98 dst tiles per core; last tile 84 rows
LAST_ROWS = M - (NT - 1) * P
MP = NT * P              # padded rows per core (12544)
# fp8e4 pair-gather (idx fetches an aligned PAIR of fp8 rows to satisfy the
# 256B elem granularity; parity-split mask columns select the half): halves
# AG SDMA time but doubles the per-tile PE accumulation chain and (at GT=2)
# adds 96 gather calls/layer — measured 1681us vs 1440-1520 baseline, with
# rel err 1.535e-2. Net negative; keep off.
GATHER_FP8 = False
PAR = 2 if GATHER_FP8 else 1
GT = 2 if GATHER_FP8 else 4   # dst tiles per group (= shared PSUM slots);
                              # GT=2 keeps mask slabs in SBUF despite PAR=2
NG = (NT + GT - 1) // GT  # 17 groups
GA = 8                   # stage-A tiles per group
QS = [0, 25, 50, 75]     # quarter starts (tiles); balanced quarters keep the
TQ = [25, 25, 25, 23]    # 4 gather queues evenly loaded (uneven = +300us)
NB = 4                   # AG quarters
# Own-core edges (src core == dst core, ~12.5%) gather from p_chunk/h_chunk
# (ready before the AllGather) via a 5th bank, so those gathers overlap the
# AG wire window instead of idling the DMA engines.
OWN_BANK = True
NBK = NB + 1 if OWN_BANK else NB   # gather banks (bank NB = own-core table)
ALIGN = 16               # per (g,b,t) capacity alignment
WIDE_GATHER = False      # dead end: gather is byte-bound; 512B descs = +55% time
AGG_BUFS = 2 if WIDE_GATHER else 4   # gbuf doubles per buf; keep SBUF constant
OUT_BF16 = True          # write output in bf16; host upcasts to f32
REPEAT = 1
SINGLE_PACKET = False    # True wedges the device (reproduced twice)
NQUEUES = 4
PAD_IDX = 0              # -1: gather skips pad rows (if ucode allows interior)

BF16 = ml_dtypes.bfloat16


def _quarter_of_tile(t):
    return np.searchsorted(np.asarray(QS), t, side="right") - 1


def _plan(counts):
    """counts: [NCORES, NG, NB, GT] edge counts. Returns static schedule."""
    U = ((counts.max(axis=0) + ALIGN - 1) // ALIGN) * ALIGN  # [NG, NB, GT]
    groups = []
    idx_off = 0   # int16 elements into the global idx stream
    ch_off = 0    # global dstloc column offset
    for g in range(NG):
        tiles = list(range(g * GT, min((g + 1) * GT, NT)))
        nt_g = len(tiles)
        blen = []      # idxs per bank gather call
        gb_choff = []  # chunk offset of each bank segment inside gbuf
        toff = []      # [b][j]: stream offset of tile j within bank b
        ch = 0
        for b in range(NBK):
            off = np.zeros(nt_g + 1, dtype=np.int64)
            off[1:] = np.cumsum(U[g, b, :nt_g])
            # round the bank segment to whole chunks so every gbuf row that a
            # mask can touch is actually written (stale rows can be NaN/Inf
            # and 0*NaN poisons the masked matmul)
            L = ((int(off[-1]) + P - 1) // P) * P
            toff.append(off)
            blen.append(L)
            gb_choff.append(ch)
            ch += L // P
        # tile-major dstloc columns; a chunk overlapping tile j contributes
        # one (bank, gbuf_chunk, dstloc_col) entry to that tile's matmul list
        tile_chunks = {t: [] for t in tiles}
        dcol = ch_off
        for j, t in enumerate(tiles):
            for b in range(NBK):
                u = int(U[g, b, j])
                if u == 0:
                    continue
                lo = int(toff[b][j]) // P
                hi = (int(toff[b][j + 1]) - 1) // P
                for c in range(lo, hi + 1):
                    tile_chunks[t].append((b, gb_choff[b] + c, dcol))
                    dcol += PAR
        groups.append(
            dict(
                tiles=tiles,
                blen=blen,
                gb_choff=gb_choff,
                toff=toff,
                CHg=ch,
                idx_off=idx_off,
                dst_off=ch_off,
                nd=dcol - ch_off,
                tile_chunks=tile_chunks,
            )
        )
        idx_off += sum(blen)
        ch_off = dcol
    return U, groups, idx_off, ch_off


def _host_prep(x, edge_index, W1, b1, W2, b2):
    x = np.asarray(x, dtype=np.float32)
    ei = np.asarray(edge_index).astype(np.int64)
    src, dst = ei[0], ei[1]
    deg = np.bincount(dst, minlength=N).astype(np.float32) + 1.0
    dinv = (1.0 / np.sqrt(deg)).astype(np.float32)

    # source bank (= quarter) and bank-local int16 index, core-major layout
    s_c = src // M
    s_l = src % M
    s_t = s_l // P
    s_q = _quarter_of_tile(s_t)
    tq_arr = np.asarray(TQ)[s_q]
    qs_arr = np.asarray(QS)[s_q]
    src16 = (s_c * tq_arr * P + (s_t - qs_arr) * P + (s_l % P)).astype(np.int16)

    d_c = dst // M
    d_l = dst % M
    d_t = d_l // P
    d_g = d_t // GT
    d_tl = d_t % GT          # tile index within group
    d_loc = (d_l % P).astype(np.float32)

    if OWN_BANK:
        own = s_c == d_c
        s_q = np.where(own, NB, s_q)
        src16 = np.where(own, s_l, src16.astype(np.int64)).astype(np.int16)
    key = ((d_c * NG + d_g) * NBK + s_q) * GT + d_tl
    counts = np.bincount(key, minlength=NCORES * NG * NBK * GT).reshape(
        NCORES, NG, NBK, GT
    )
    order = np.argsort(key, kind="stable")
    starts = np.zeros(NCORES * NG * NBK * GT + 1, dtype=np.int64)
    starts[1:] = np.cumsum(counts.reshape(-1))

    U, groups, TOTIDX, TOTCH = _plan(counts)

    w1_bf = np.asarray(W1, np.float32).astype(BF16)
    w2_bf = np.asarray(W2, np.float32).astype(BF16)
    b1_nz = bool(np.any(np.asarray(b1)))
    b2_nz = bool(np.any(np.asarray(b2)))

    xT = (x * dinv[:, None]).T.astype(BF16)  # [256, N], row-scaled

    in_maps = []
    for c in range(NCORES):
        idx_all = np.full(TOTIDX, PAD_IDX, dtype=np.int16)
        dst_cols = np.full((TOTCH, P), -1.0, dtype=np.float32)
        for g_i, g in enumerate(groups):
            seg = g["idx_off"]
            nt_g = len(g["tiles"])
            for b in range(NBK):
                for j in range(nt_g):
                    k = ((c * NG + g_i) * NBK + b) * GT + j
                    n = counts[c, g_i, b, j]
                    if n:
                        e = order[starts[k] : starts[k] + n]
                        pos = seg + int(g["toff"][b][j])
                        idx_all[pos : pos + n] = (
                            src16[e] >> 1 if PAR == 2 else src16[e]
                        )
                seg += g["blen"][b]
            # dstloc columns (tile-major)
            for j, t in enumerate(g["tiles"]):
                for b in range(NBK):
                    u = int(U[g_i, b, j])
                    if u == 0:
                        continue
                    k = ((c * NG + g_i) * NBK + b) * GT + j
                    n = counts[c, g_i, b, j]
                    t0 = int(g["toff"][b][j])
                    # chunks overlapping this tile within bank b
                    lo = t0 // P
                    hi = (t0 + u - 1) // P
                    # find dstloc cols assigned to (t, b) in tile_chunks order
                    cols = [
                        col for (bb, chk, col) in g["tile_chunks"][t] if bb == b
                    ]
                    assert len(cols) == hi - lo + 1
                    if n:
                        e = order[starts[k] : starts[k] + n]
                        vals = d_loc[e]
                        par = (src16[e] & 1).astype(np.int64)
                        for ci, cchunk in enumerate(range(lo, hi + 1)):
                            col = cols[ci]
                            # positions of this chunk within the bank stream
                            p0 = cchunk * P
                            # rows of this chunk belonging to tile j
                            r_lo = max(t0, p0)
                            r_hi = min(t0 + n, p0 + P)
                            if r_hi > r_lo:
                                vsl = vals[r_lo - t0 : r_hi - t0]
                                if PAR == 1:
                                    dst_cols[col, r_lo - p0 : r_hi - p0] = vsl
                                else:
                                    # parity-split: column col matches rows
                                    # whose src was the even half of its fp8
                                    # pair, col+1 the odd half
                                    psl = par[r_lo - t0 : r_hi - t0]
                                    dst_cols[col, r_lo - p0 : r_hi - p0] = (
                                        np.where(psl == 0, vsl, -1.0)
                                    )
                                    dst_cols[col + 1, r_lo - p0 : r_hi - p0] = (
                                        np.where(psl == 1, vsl, -1.0)
                                    )
        # wrap idx stream per (g,b): idx k -> partition k%16, col k//16
        blocks = []
        for g in groups:
            seg = g["idx_off"]
            for b in range(NBK):
                L = g["blen"][b]
                if L:
                    blocks.append(idx_all[seg : seg + L].reshape(-1, 16).T)
                seg += L
        idxw = np.tile(np.hstack(blocks), (8, 1))  # [128, TOTIDX//16]

        dv = dinv[c * M : (c + 1) * M]
        dvp = np.concatenate([dv, np.ones(MP - M, np.float32)])
        dvp2 = dvp * dvp
        sq = np.sqrt(np.concatenate([deg[c * M : (c + 1) * M], np.ones(MP - M, np.float32)]))
        in_maps.append(
            {
                "xT": np.ascontiguousarray(xT[:, c * M : (c + 1) * M]),
                "idx": idxw,
                "dstloc": np.ascontiguousarray(dst_cols.T).astype(BF16),
                "dinv": np.ascontiguousarray(dvp.reshape(NT, P).T),
                "dinv2": np.ascontiguousarray(dvp2.reshape(NT, P).T),
                "sqd": sq.reshape(1, NT, P).astype(BF16),
                "iota": np.tile(np.arange(P, dtype=np.float32), (P, 1))
                .astype(BF16)
                .reshape(P, 1, P),
                "W1": w1_bf,
                "W2": w2_bf,
                "b1row": np.asarray(b1, np.float32).reshape(1, HID).astype(BF16),
                "b2row": np.asarray(b2, np.float32).reshape(1, OUTD).astype(BF16),
            }
        )
    return in_maps, U, groups, TOTIDX, TOTCH, b1_nz, b2_nz


ABLATE = frozenset()


def _build_nc(groups, TOTIDX, TOTCH, b1_nz, b2_nz, ablate=None):
    ablate = ABLATE if ablate is None else ablate
    import concourse.bacc as bacc
    import concourse.mybir as mybir
    import concourse.tile as tile
    from concourse.masks import make_identity

    F32 = mybir.dt.float32
    BF = mybir.dt.bfloat16
    F8 = mybir.dt.float8e4
    GD = F8 if GATHER_FP8 else BF   # gather-table dtype (banks/chunks/gbuf)
    I16 = mybir.dt.int16
    AOP = mybir.AluOpType
    ACT_RELU = mybir.ActivationFunctionType.Relu
    ACT_COPY = mybir.ActivationFunctionType.Copy

    nc = bacc.Bacc(
        "TRN2", target_bir_lowering=False, num_devices=NCORES, num_swdge_queues=NQUEUES
    )
    xT_d = nc.dram_tensor("xT", [IND, M], BF, kind="ExternalInput")
    idx_d = nc.dram_tensor("idx", [P, TOTIDX // 16], I16, kind="ExternalInput")
    dst_d = nc.dram_tensor("dstloc", [P, TOTCH], BF, kind="ExternalInput")
    dinv_d = nc.dram_tensor("dinv", [P, NT], F32, kind="ExternalInput")
    dinv2_d = nc.dram_tensor("dinv2", [P, NT], F32, kind="ExternalInput")
    sqd_d = nc.dram_tensor("sqd", [1, NT, P], BF, kind="ExternalInput")
    iota_d = nc.dram_tensor("iota", [P, 1, P], BF, kind="ExternalInput")
    w1_d = nc.dram_tensor("W1", [IND, HID], BF, kind="ExternalInput")
    w2_d = nc.dram_tensor("W2", [HID, OUTD], BF, kind="ExternalInput")
    b1_d = nc.dram_tensor("b1row", [1, HID], BF, kind="ExternalInput")
    b2_d = nc.dram_tensor("b2row", [1, OUTD], BF, kind="ExternalInput")
    out_d = nc.dram_tensor("out", [M, OUTD], BF if OUT_BF16 else F32,
                           kind="ExternalOutput")

    CHMAX = max(g["CHg"] for g in groups)
    NDMAX = max(g["nd"] for g in groups)
    NCHMAX = max(len(ck) for g in groups for ck in g["tile_chunks"].values())
    IDXWMAX = max(sum(g["blen"]) for g in groups) // 16

    # quarter boundaries in groups: AG_q can fire once group `qgrp[q]` done
    qgrp = [min((QS[q] + TQ[q] + GT - 1) // GT - 1, NG - 1) for q in range(NB)]

    with tile.TileContext(nc) as tc:
        with (
            tc.tile_pool(name="dram", bufs=1, space="DRAM") as dpool,
            tc.tile_pool(name="const", bufs=1) as cp,
            tc.tile_pool(name="resid", bufs=1) as rp,
        ):
            p_chunk = dpool.tile([MP, HID], GD)
            h_chunk = dpool.tile([MP, HID], GD)
            # +P pad rows so a 512B (2-row) descriptor at the last real row
            # stays in bounds; pad content is garbage and never consumed
            PADR = P if WIDE_GATHER else 0
            p_bank = [
                dpool.tile([NCORES * TQ[q] * P + PADR, HID], GD,
                           addr_space="Shared", name=f"p_bank{q}")
                for q in range(NB)
            ]
            h_bank = [
                dpool.tile([NCORES * TQ[q] * P + PADR, HID], GD,
                           addr_space="Shared", name=f"h_bank{q}")
                for q in range(NB)
            ]

            w1a = cp.tile([P, HID], BF)
            w1b = cp.tile([P, HID], BF)
            w2s = cp.tile([HID, OUTD], BF)
            b1r = cp.tile([1, HID], BF)
            b2r = cp.tile([1, OUTD], BF)
            sqd = cp.tile([1, NT, P], BF)
            iota = cp.tile([P, 1, P], BF)
            dinv = cp.tile([P, NT], F32)
            dinv2 = cp.tile([P, NT], F32)
            ident = cp.tile([P, P], BF)
            nc.sync.dma_start(w1a[:], w1_d[0:P, :])
            nc.sync.dma_start(w1b[:], w1_d[P:IND, :])
            nc.sync.dma_start(w2s[:], w2_d[:])
            nc.sync.dma_start(b1r[:], b1_d[:])
            nc.sync.dma_start(b2r[:], b2_d[:])
            nc.sync.dma_start(sqd[:], sqd_d[:])
            nc.sync.dma_start(iota[:], iota_d[:])
            nc.sync.dma_start(dinv[:], dinv_d[:])
            nc.sync.dma_start(dinv2[:], dinv2_d[:])
            make_identity(nc, ident[:])

            # resident per-core feature copies (also DMA'd to DRAM for AG)
            p_sb = rp.tile([P, NT, HID], BF)
            h_sb = rp.tile([P, NT, HID], BF)
            # last tile only has LAST_ROWS valid rows; zero the tail lanes
            nc.vector.memset(p_sb[:, NT - 1, :], 0.0)
            nc.vector.memset(h_sb[:, NT - 1, :], 0.0)
            if GATHER_FP8:
                # fp8 copies feeding the AG chunks (self-loop stays bf16)
                p8_sb = rp.tile([P, NT, HID], GD)
                h8_sb = rp.tile([P, NT, HID], GD)
                nc.vector.memset(p8_sb[:, NT - 1, :], 0.0)
                nc.vector.memset(h8_sb[:, NT - 1, :], 0.0)
            else:
                p8_sb, h8_sb = p_sb, h_sb

            def emit_ag(q, chunk, banks):
                if "ag" in ablate:
                    return
                q0 = QS[q] * P
                q1 = (QS[q] + TQ[q]) * P
                nc.gpsimd.collective_compute(
                    "AllGather", mybir.AluOpType.bypass,
                    replica_groups=[list(range(NCORES))],
                    ins=[chunk[q0:q1, :].opt()],
                    outs=[banks[q][0 : NCORES * TQ[q] * P, :].opt()],
                )

            # ---------------- stage A: P' = (dinv.x) @ W1 ----------------
            NGA = (NT + GA - 1) // GA
            qgrpA = [min((QS[q] + TQ[q] + GA - 1) // GA - 1, NGA - 1)
                     for q in range(NB)]
            for _rep in range(REPEAT):
              with (
                  tc.tile_pool(name="sa", bufs=2) as sa,
                  tc.tile_pool(name="psA", bufs=2, space="PSUM") as psA,
              ):
                  for g_i in range(NGA):
                      tiles = list(range(g_i * GA, min((g_i + 1) * GA, NT)))
                      c0 = tiles[0] * P
                      c1 = min(c0 + GA * P, M)
                      cols = c1 - c0
                      xa = sa.tile([P, GA * P], BF, tag="xa")
                      xb = sa.tile([P, GA * P], BF, tag="xb")
                      nc.sync.dma_start(xa[:, :cols], xT_d[0:P, c0:c1])
                      nc.sync.dma_start(xb[:, :cols], xT_d[P:IND, c0:c1])
                      for j, t in enumerate(tiles):
                          rows = P if t < NT - 1 else LAST_ROWS
                          ps = psA.tile([P, HID], mybir.dt.float32, tag="psA")
                          nc.tensor.matmul(
                              ps[:rows, :], lhsT=xa[:, j * P : j * P + rows],
                              rhs=w1a[:], start=True, stop=False,
                          )
                          nc.tensor.matmul(
                              ps[:rows, :], lhsT=xb[:, j * P : j * P + rows],
                              rhs=w1b[:], start=False, stop=True,
                          )
                          nc.scalar.activation(
                              p_sb[:rows, t, :], ps[:rows, :], ACT_COPY, bias=0.0,
                          )
                          if GATHER_FP8:
                              nc.scalar.activation(
                                  p8_sb[:rows, t, :], ps[:rows, :], ACT_COPY,
                                  bias=0.0,
                              )
                      # always full tiles into the padded chunk
                      nc.sync.dma_start(
                          p_chunk[c0 : c0 + len(tiles) * P, :].rearrange(
                              "(a p) f -> p a f", p=P
                          ),
                          p8_sb[:, tiles[0] : tiles[0] + len(tiles), :],
                      )
                      for q in range(NB):
                          if qgrpA[q] == g_i:
                              emit_ag(q, p_chunk, p_bank)

              # ------------- aggregation layers -------------
              with (
                  tc.tile_pool(name="agg", bufs=AGG_BUFS) as ag,
                  tc.tile_pool(name="mskp", bufs=2) as mskp,
                  tc.tile_pool(name="oslabp", bufs=2) as oslabp,
                  tc.tile_pool(name="ttp", bufs=3) as ttp,
                  tc.tile_pool(name="psAg", bufs=GT, space="PSUM") as psAg,
                  tc.tile_pool(name="psF", bufs=2, space="PSUM") as psF,
              ):
                  for layer in (0, 1):
                      banks = p_bank if layer == 0 else h_bank
                      if OWN_BANK:
                          banks = banks + [p_chunk if layer == 0 else h_chunk]
                      own_sb = p_sb if layer == 0 else h_sb
                      for g_i, g in enumerate(groups):
                          W16 = sum(g["blen"]) // 16
                          io = g["idx_off"] // 16
                          idxs = ag.tile([P, IDXWMAX], I16, tag="idxs")
                          nc.sync.dma_start(idxs[:, :W16], idx_d[:, io : io + W16])
                          dstl = ag.tile([P, NDMAX], BF, tag="dstl")
                          nc.sync.dma_start(
                              dstl[:, : g["nd"]],
                              dst_d[:, g["dst_off"] : g["dst_off"] + g["nd"]],
                          )
                          GW = (PAR * HID if GATHER_FP8
                                else (2 * HID if WIDE_GATHER else HID))
                          gbuf = ag.tile([P, CHMAX, GW], GD, tag="gbuf")
                          boff = 0
                          for b in range(NBK):
                              L = g["blen"][b]
                              if L == 0 or "gather" in ablate:
                                  if L and "gather" in ablate and b == 0:
                                      nc.vector.memset(gbuf[:, :, 0:2], 0.0)
                                  continue
                              nch_b = (L + P - 1) // P
                              if WIDE_GATHER:
                                  # 512B window starting at each row: overlapping
                                  # strided view, elem_step = one row
                                  bap = banks[b][:]
                                  src = type(bap)(
                                      tensor=bap.tensor,
                                      offset=bap.offset,
                                      ap=[[HID, NCORES * TQ[b] * P], [1, GW]],
                                  )
                                  nc.gpsimd.dma_gather(
                                      gbuf[:, g["gb_choff"][b] : g["gb_choff"][b] + nch_b, :],
                                      src,
                                      idxs[:, boff : boff + L // 16],
                                      L, L, GW, elem_step=HID,
                                      queue_num=b % NQUEUES,
                                      single_packet=SINGLE_PACKET,
                                  )
                              else:
                                  if GATHER_FP8:
                                      # idx addresses an aligned PAIR of fp8
                                      # rows: 256B elems satisfy the ucode
                                      # granularity; masks select the half
                                      src_ap = banks[b][:].rearrange(
                                          "(a two) f -> a (two f)", two=2
                                      )
                                      esz = PAR * HID
                                  else:
                                      src_ap = banks[b][:]
                                      esz = HID
                                  nc.gpsimd.dma_gather(
                                      gbuf[:, g["gb_choff"][b] : g["gb_choff"][b] + nch_b, :],
                                      src_ap,
                                      idxs[:, boff : boff + L // 16],
                                      L, L, esz, queue_num=b % NQUEUES,
                                      single_packet=SINGLE_PACKET,
                                  )
                              boff += L // 16
                          oslab = (
                              None
                              if layer == 0
                              else oslabp.tile([P, GT, OUTD], BF if OUT_BF16 else mybir.dt.float32,
                                           tag="oslab")
                          )
                          # one mask slab per group: all tiles' one-hot columns
                          nd = g["nd"]
                          msk = mskp.tile([P, NDMAX, P], BF, tag="msk")
                          if "st3" in ablate:
                              nc.vector.memset(msk[:, :2, 0:2], 0.0)
                          else:
                              nc.vector.tensor_tensor(
                                  out=msk[:, :nd, :],
                                  in0=iota[:].to_broadcast([P, nd, P]),
                                  in1=dstl[:, :nd]
                                  .rearrange("p (a b) -> p a b", b=1)
                                  .to_broadcast([P, nd, P]),
                                  op=mybir.AluOpType.is_equal,
                              )
                          # pass -1: self-loop (+bias) matmuls
                          tiles = g["tiles"]
                          state = {}
                          for j, t in enumerate(tiles):
                              chunks = g["tile_chunks"][t]
                              if "mm" in ablate:
                                  chunks = chunks[:1]
                              d0 = g["tile_chunks"][t][0][2] - g["dst_off"] if chunks else 0
                              ps = psAg.tile([P, P], mybir.dt.float32, tag="psAg",
                                             padded_shape=[P, 512])
                              l0bias = layer == 0 and b1_nz
                              if layer == 0:
                                  nc.tensor.matmul(
                                      ps[:], lhsT=ident[:], rhs=own_sb[:, t, :],
                                      start=True, stop=(not chunks and not l0bias),
                                  )
                              else:
                                  nc.tensor.matmul(
                                      ps[:], lhsT=own_sb[:, t, :], rhs=ident[:],
                                      start=True, stop=(not chunks),
                                  )
                              if l0bias:
                                  nc.tensor.matmul(
                                      ps[:], lhsT=sqd[0:1, t, :], rhs=b1r[:],
                                      start=False, stop=(not chunks),
                                  )
                              state[t] = (ps, d0, chunks, len(chunks))
                          # bank passes: chunk matmuls in bank arrival order
                          done_cnt = {t: 0 for t in tiles}

                          def finalize(j, t, ps):
                              rows = P if t < NT - 1 else LAST_ROWS
                              if layer == 0:
                                  nc.scalar.activation(
                                      h_sb[:rows, t, :], ps[:rows, :], ACT_RELU,
                                      scale=dinv2[:rows, t : t + 1],
                                  )
                                  if GATHER_FP8:
                                      nc.scalar.activation(
                                          h8_sb[:rows, t, :], ps[:rows, :],
                                          ACT_RELU,
                                          scale=dinv2[:rows, t : t + 1],
                                      )
                              else:
                                  tt = ttp.tile([P, P], BF, tag="tt")
                                  nc.scalar.activation(
                                      tt[:], ps[:], ACT_COPY, bias=0.0,
                                  )
                                  pf = psF.tile([P, OUTD], mybir.dt.float32, tag="psF")
                                  nc.tensor.matmul(
                                      pf[:rows, :], lhsT=tt[:, :rows], rhs=w2s[:],
                                      start=True, stop=(not b2_nz),
                                  )
                                  if b2_nz:
                                      nc.tensor.matmul(
                                          pf[:rows, :], lhsT=sqd[0:1, t, :rows],
                                          rhs=b2r[:], start=False, stop=True,
                                      )
                                  nc.scalar.activation(
                                      oslab[:rows, j, :], pf[:rows, :], ACT_COPY,
                                      scale=dinv[:rows, t : t + 1],
                                  )

                          for j, t in enumerate(tiles):
                              if not state[t][2]:
                                  finalize(j, t, state[t][0])
                          for b in range(NBK):
                              for j, t in enumerate(tiles):
                                  ps, d0, chunks, ntot = state[t]
                                  for k, (eb, chk, dc) in enumerate(chunks):
                                      if eb != b:
                                          continue
                                      done_cnt[t] += 1
                                      last = done_cnt[t] == len(chunks)
                                      col0 = dc - g["dst_off"]
                                      for pi in range(PAR):
                                          stop_now = last and pi == PAR - 1
                                          gsl = gbuf[:, chk, pi * HID : (pi + 1) * HID]
                                          if layer == 0:
                                              nc.tensor.matmul(
                                                  ps[:], lhsT=msk[:, col0 + pi, :],
                                                  rhs=gsl,
                                                  start=False, stop=stop_now,
                                              )
                                          else:
                                              nc.tensor.matmul(
                                                  ps[:], lhsT=gsl,
                                                  rhs=msk[:, col0 + pi, :],
                                                  start=False, stop=stop_now,
                                              )
                                      if last:
                                          finalize(j, t, ps)
                          # write the group's output rows
                          if layer == 0:
                              c0 = g["tiles"][0] * P
                              nc.sync.dma_start(
                                  h_chunk[c0 : c0 + len(g["tiles"]) * P, :].rearrange(
                                      "(a p) f -> p a f", p=P
                                  ),
                                  h8_sb[:, g["tiles"][0] : g["tiles"][0] + len(g["tiles"]), :],
                              )
                              for q in range(NB):
                                  if qgrp[q] == g_i:
                                      emit_ag(q, h_chunk, h_bank)
                          else:
                              c0 = g["tiles"][0] * P
                              c1 = min(c0 + GT * P, M)
                              if c1 - c0 == len(g["tiles"]) * P:
                                  nc.sync.dma_start(
                                      out_d[c0:c1, :].rearrange("(a p) f -> p a f", p=P),
                                      oslab[:, : len(g["tiles"]), :],
                                  )
                              else:
                                  for j, t in enumerate(g["tiles"]):
                                      rows = P if t < NT - 1 else LAST_ROWS
                                      r0 = c0 + j * P
                                      nc.sync.dma_start(
                                          out_d[r0 : r0 + rows, :], oslab[:rows, j, :]
                                      )
    nc.compile()
    # Align each gather's SWDGE queue with its round-robin DMASW semaphore
    # lane (lane i -> queue i%NQUEUES). The tile scheduler may reorder
    # gathers, and a lane shared by two queues can release a consumer early
    # (queue FIFOs are unordered relative to each other). Post-patch the
    # queue_num so lane and queue FIFO order always agree.
    from concourse.bass_isa import AnyDMAInstruction
    lane = 0
    for bb in nc.m.functions[0].blocks:
        for inst in bb.instructions:
            if inst.engine == mybir.EngineType.Pool and isinstance(
                inst, AnyDMAInstruction
            ):
                if hasattr(inst, "queue_num"):
                    inst.queue_num = lane % NQUEUES
                lane = (lane + 1) % 8
    return nc


_CACHE = {}


def _get_compiled(x, edge_index, W1, b1, W2, b2):
    in_maps, U, groups, TOTIDX, TOTCH, b1_nz, b2_nz = _host_prep(
        x, edge_index, W1, b1, W2, b2
    )
    key = (TOTIDX, TOTCH, ABLATE, GT, AGG_BUFS, REPEAT, b1_nz, b2_nz,
           SINGLE_PACKET, NQUEUES, PAD_IDX, OUT_BF16, ALIGN, GATHER_FP8,
           WIDE_GATHER, OWN_BANK,
           tuple(int(v) for v in np.asarray(U).reshape(-1)[:64]))
    if key not in _CACHE:
        _CACHE[key] = _build_nc(groups, TOTIDX, TOTCH, b1_nz, b2_nz)
    return _CACHE[key], in_maps


def kernel(x, edge_index, W1, b1, W2, b2):
    from concourse.bass_utils import run_bass_kernel_spmd

    nc, in_maps = _get_compiled(x, edge_index, W1, b1, W2, b2)
    res = run_bass_kernel_spmd(nc, in_maps, core_ids=list(range(NCORES)))
    out = np.concatenate([res.results[c]["out"] for c in range(NCORES)], axis=0)
    return np.ascontiguousarray(out.astype(np.float32))



# revision 46
# speedup vs baseline: 1.0718x; 1.0718x over previous
"""Two-layer GCN (PyG GCNConv style) on 8 Trainium2 NeuronCores — v2.

Math (reference):
    src,dst += self-loops; deg = indeg(dst)+1 ; dinv = deg^-1/2
    norm_e  = dinv[src]*dinv[dst]
    h  = relu( scatter(norm * (x@W1)[src] -> dst) + b1 )
    out =      scatter(norm * (h@W2)[src] -> dst) + b2

Factorization: with P' = dinv (.) (x@W1) = (dinv (.) x)@W1,
    h' = dinv (.) h = relu( dinv^2 * (agg + P'[d] + sqrt(deg)*b1-term) )
    out[d] = dinv[d] * ( (agg2 + h'[d]) @ W2 ) + b2
so scatters are pure segment-sums of gathered rows done as one-hot
matmuls on the PE; epilogues ride the ACT engine (relu/copy with
per-partition scale); self-loop and bias terms are extra PE matmuls.

v2 changes vs v1:
  - AllGather split into 4 tile-quarters (one per gather bank) so the
    collective pipelines with the gather stream; layer-2's AllGather
    overlaps layer-1 aggregation.
  - Feature tables are stored core-major per quarter; gather banks ==
    quarter tables.
  - Per (group,bank,tile) capacities aligned to 32 (not 128-per-tile),
    chunks may span tiles (dual dstloc columns); ~6% padding vs ~25%.
  - GT=8 dst tiles per group: half the dma_gather calls.
  - Epilogues on ACT (scale=dinv^2 / dinv), freeing the DVE for the
    one-hot builds; self-loop + bias folded into PSUM accumulation.
"""

import numpy as np
import ml_dtypes

N = 100000
NCORES = 8
M = N // NCORES          # 12500 nodes per core
IND, HID, OUTD = 256, 128, 237
P = 128
NT = (M + P - 1) // P    # 98 dst tiles per core; last tile 84 rows
LAST_ROWS = M - (NT - 1) * P
MP = NT * P              # padded rows per core (12544)
# fp8e4 pair-gather (idx fetches an aligned PAIR of fp8 rows to satisfy the
# 256B elem granularity; parity-split mask columns select the half): halves
# AG SDMA time but doubles the per-tile PE accumulation chain and (at GT=2)
# adds 96 gather calls/layer — measured 1681us vs 1440-1520 baseline, with
# rel err 1.535e-2. Net negative; keep off.
GATHER_FP8 = False
PAR = 2 if GATHER_FP8 else 1
GT = 2 if GATHER_FP8 else 4   # dst tiles per group (= shared PSUM slots);
                              # GT=2 keeps mask slabs in SBUF despite PAR=2
NG = (NT + GT - 1) // GT  # 17 groups
GA = 8                   # stage-A tiles per group
QS = [0, 25, 50, 75]     # quarter starts (tiles); balanced quarters keep the
TQ = [25, 25, 25, 23]    # 4 gather queues evenly loaded (uneven = +300us)
NB = 4                   # AG quarters
# Own-core edges (src core == dst core, ~12.5%) gather from p_chunk/h_chunk
# (ready before the AllGather) via a 5th bank, so those gathers overlap the
# AG wire window instead of idling the DMA engines.
# Measured 1570us vs 1497 baseline: the AGG_BUFS pool gates the overlap
# window to ~4 groups (~10us hoistable), and +1 call/group cancels it.
OWN_BANK = False
NBK = NB + 1 if OWN_BANK else NB   # gather banks (bank NB = own-core table)
ALIGN = 16               # per (g,b,t) capacity alignment
WIDE_GATHER = False      # dead end: gather is byte-bound; 512B descs = +55% time
AGG_BUFS = 2 if WIDE_GATHER else 4   # gbuf doubles per buf; keep SBUF constant
OUT_BF16 = True          # write output in bf16; host upcasts to f32
REPEAT = 1
SINGLE_PACKET = False    # True wedges the device (reproduced twice)
NQUEUES = 4
PAD_IDX = 0              # -1: gather skips pad rows (if ucode allows interior)

BF16 = ml_dtypes.bfloat16


def _quarter_of_tile(t):
    return np.searchsorted(np.asarray(QS), t, side="right") - 1


def _plan(counts):
    """counts: [NCORES, NG, NB, GT] edge counts. Returns static schedule."""
    U = ((counts.max(axis=0) + ALIGN - 1) // ALIGN) * ALIGN  # [NG, NB, GT]
    groups = []
    idx_off = 0   # int16 elements into the global idx stream
    ch_off = 0    # global dstloc column offset
    for g in range(NG):
        tiles = list(range(g * GT, min((g + 1) * GT, NT)))
        nt_g = len(tiles)
        blen = []      # idxs per bank gather call
        gb_choff = []  # chunk offset of each bank segment inside gbuf
        toff = []      # [b][j]: stream offset of tile j within bank b
        ch = 0
        for b in range(NBK):
            off = np.zeros(nt_g + 1, dtype=np.int64)
            off[1:] = np.cumsum(U[g, b, :nt_g])
            # round the bank segment to whole chunks so every gbuf row that a
            # mask can touch is actually written (stale rows can be NaN/Inf
            # and 0*NaN poisons the masked matmul)
            L = ((int(off[-1]) + P - 1) // P) * P
            toff.append(off)
            blen.append(L)
            gb_choff.append(ch)
            ch += L // P
        # tile-major dstloc columns; a chunk overlapping tile j contributes
        # one (bank, gbuf_chunk, dstloc_col) entry to that tile's matmul list
        tile_chunks = {t: [] for t in tiles}
        dcol = ch_off
        for j, t in enumerate(tiles):
            for b in range(NBK):
                u = int(U[g, b, j])
                if u == 0:
                    continue
                lo = int(toff[b][j]) // P
                hi = (int(toff[b][j + 1]) - 1) // P
                for c in range(lo, hi + 1):
                    tile_chunks[t].append((b, gb_choff[b] + c, dcol))
                    dcol += PAR
        groups.append(
            dict(
                tiles=tiles,
                blen=blen,
                gb_choff=gb_choff,
                toff=toff,
                CHg=ch,
                idx_off=idx_off,
                dst_off=ch_off,
                nd=dcol - ch_off,
                tile_chunks=tile_chunks,
            )
        )
        idx_off += sum(blen)
        ch_off = dcol
    return U, groups, idx_off, ch_off


def _host_prep(x, edge_index, W1, b1, W2, b2):
    x = np.asarray(x, dtype=np.float32)
    ei = np.asarray(edge_index).astype(np.int64)
    src, dst = ei[0], ei[1]
    deg = np.bincount(dst, minlength=N).astype(np.float32) + 1.0
    dinv = (1.0 / np.sqrt(deg)).astype(np.float32)

    # source bank (= quarter) and bank-local int16 index, core-major layout
    s_c = src // M
    s_l = src % M
    s_t = s_l // P
    s_q = _quarter_of_tile(s_t)
    tq_arr = np.asarray(TQ)[s_q]
    qs_arr = np.asarray(QS)[s_q]
    src16 = (s_c * tq_arr * P + (s_t - qs_arr) * P + (s_l % P)).astype(np.int16)

    d_c = dst // M
    d_l = dst % M
    d_t = d_l // P
    d_g = d_t // GT
    d_tl = d_t % GT          # tile index within group
    d_loc = (d_l % P).astype(np.float32)

    if OWN_BANK:
        own = s_c == d_c
        s_q = np.where(own, NB, s_q)
        src16 = np.where(own, s_l, src16.astype(np.int64)).astype(np.int16)
    key = ((d_c * NG + d_g) * NBK + s_q) * GT + d_tl
    counts = np.bincount(key, minlength=NCORES * NG * NBK * GT).reshape(
        NCORES, NG, NBK, GT
    )
    order = np.argsort(key, kind="stable")
    starts = np.zeros(NCORES * NG * NBK * GT + 1, dtype=np.int64)
    starts[1:] = np.cumsum(counts.reshape(-1))

    U, groups, TOTIDX, TOTCH = _plan(counts)

    w1_bf = np.asarray(W1, np.float32).astype(BF16)
    w2_bf = np.asarray(W2, np.float32).astype(BF16)
    b1_nz = bool(np.any(np.asarray(b1)))
    b2_nz = bool(np.any(np.asarray(b2)))

    xT = (x * dinv[:, None]).T.astype(BF16)  # [256, N], row-scaled

    in_maps = []
    for c in range(NCORES):
        idx_all = np.full(TOTIDX, PAD_IDX, dtype=np.int16)
        dst_cols = np.full((TOTCH, P), -1.0, dtype=np.float32)
        for g_i, g in enumerate(groups):
            seg = g["idx_off"]
            nt_g = len(g["tiles"])
            for b in range(NBK):
                for j in range(nt_g):
                    k = ((c * NG + g_i) * NBK + b) * GT + j
                    n = counts[c, g_i, b, j]
                    if n:
                        e = order[starts[k] : starts[k] + n]
                        pos = seg + int(g["toff"][b][j])
                        idx_all[pos : pos + n] = (
                            src16[e] >> 1 if PAR == 2 else src16[e]
                        )
                seg += g["blen"][b]
            # dstloc columns (tile-major)
            for j, t in enumerate(g["tiles"]):
                for b in range(NBK):
                    u = int(U[g_i, b, j])
                    if u == 0:
                        continue
                    k = ((c * NG + g_i) * NBK + b) * GT + j
                    n = counts[c, g_i, b, j]
                    t0 = int(g["toff"][b][j])
                    # chunks overlapping this tile within bank b
                    lo = t0 // P
                    hi = (t0 + u - 1) // P
                    # find dstloc cols assigned to (t, b) in tile_chunks order
                    cols = [
                        col for (bb, chk, col) in g["tile_chunks"][t] if bb == b
                    ]
                    assert len(cols) == hi - lo + 1
                    if n:
                        e = order[starts[k] : starts[k] + n]
                        vals = d_loc[e]
                        par = (src16[e] & 1).astype(np.int64)
                        for ci, cchunk in enumerate(range(lo, hi + 1)):
                            col = cols[ci]
                            # positions of this chunk within the bank stream
                            p0 = cchunk * P
                            # rows of this chunk belonging to tile j
                            r_lo = max(t0, p0)
                            r_hi = min(t0 + n, p0 + P)
                            if r_hi > r_lo:
                                vsl = vals[r_lo - t0 : r_hi - t0]
                                if PAR == 1:
                                    dst_cols[col, r_lo - p0 : r_hi - p0] = vsl
                                else:
                                    # parity-split: column col matches rows
                                    # whose src was the even half of its fp8
                                    # pair, col+1 the odd half
                                    psl = par[r_lo - t0 : r_hi - t0]
                                    dst_cols[col, r_lo - p0 : r_hi - p0] = (
                                        np.where(psl == 0, vsl, -1.0)
                                    )
                                    dst_cols[col + 1, r_lo - p0 : r_hi - p0] = (
                                        np.where(psl == 1, vsl, -1.0)
                                    )
        # wrap idx stream per (g,b): idx k -> partition k%16, col k//16
        blocks = []
        for g in groups:
            seg = g["idx_off"]
            for b in range(NBK):
                L = g["blen"][b]
                if L:
                    blocks.append(idx_all[seg : seg + L].reshape(-1, 16).T)
                seg += L
        idxw = np.tile(np.hstack(blocks), (8, 1))  # [128, TOTIDX//16]

        dv = dinv[c * M : (c + 1) * M]
        dvp = np.concatenate([dv, np.ones(MP - M, np.float32)])
        dvp2 = dvp * dvp
        sq = np.sqrt(np.concatenate([deg[c * M : (c + 1) * M], np.ones(MP - M, np.float32)]))
        in_maps.append(
            {
                "xT": np.ascontiguousarray(xT[:, c * M : (c + 1) * M]),
                "idx": idxw,
                "dstloc": np.ascontiguousarray(dst_cols.T).astype(BF16),
                "dinv": np.ascontiguousarray(dvp.reshape(NT, P).T),
                "dinv2": np.ascontiguousarray(dvp2.reshape(NT, P).T),
                "sqd": sq.reshape(1, NT, P).astype(BF16),
                "iota": np.tile(np.arange(P, dtype=np.float32), (P, 1))
                .astype(BF16)
                .reshape(P, 1, P),
                "W1": w1_bf,
                "W2": w2_bf,
                "b1row": np.asarray(b1, np.float32).reshape(1, HID).astype(BF16),
                "b2row": np.asarray(b2, np.float32).reshape(1, OUTD).astype(BF16),
            }
        )
    return in_maps, U, groups, TOTIDX, TOTCH, b1_nz, b2_nz


ABLATE = frozenset()


def _build_nc(groups, TOTIDX, TOTCH, b1_nz, b2_nz, ablate=None):
    ablate = ABLATE if ablate is None else ablate
    import concourse.bacc as bacc
    import concourse.mybir as mybir
    import concourse.tile as tile
    from concourse.masks import make_identity

    F32 = mybir.dt.float32
    BF = mybir.dt.bfloat16
    F8 = mybir.dt.float8e4
    GD = F8 if GATHER_FP8 else BF   # gather-table dtype (banks/chunks/gbuf)
    I16 = mybir.dt.int16
    AOP = mybir.AluOpType
    ACT_RELU = mybir.ActivationFunctionType.Relu
    ACT_COPY = mybir.ActivationFunctionType.Copy

    nc = bacc.Bacc(
        "TRN2", target_bir_lowering=False, num_devices=NCORES, num_swdge_queues=NQUEUES
    )
    xT_d = nc.dram_tensor("xT", [IND, M], BF, kind="ExternalInput")
    idx_d = nc.dram_tensor("idx", [P, TOTIDX // 16], I16, kind="ExternalInput")
    dst_d = nc.dram_tensor("dstloc", [P, TOTCH], BF, kind="ExternalInput")
    dinv_d = nc.dram_tensor("dinv", [P, NT], F32, kind="ExternalInput")
    dinv2_d = nc.dram_tensor("dinv2", [P, NT], F32, kind="ExternalInput")
    sqd_d = nc.dram_tensor("sqd", [1, NT, P], BF, kind="ExternalInput")
    iota_d = nc.dram_tensor("iota", [P, 1, P], BF, kind="ExternalInput")
    w1_d = nc.dram_tensor("W1", [IND, HID], BF, kind="ExternalInput")
    w2_d = nc.dram_tensor("W2", [HID, OUTD], BF, kind="ExternalInput")
    b1_d = nc.dram_tensor("b1row", [1, HID], BF, kind="ExternalInput")
    b2_d = nc.dram_tensor("b2row", [1, OUTD], BF, kind="ExternalInput")
    out_d = nc.dram_tensor("out", [M, OUTD], BF if OUT_BF16 else F32,
                           kind="ExternalOutput")

    CHMAX = max(g["CHg"] for g in groups)
    NDMAX = max(g["nd"] for g in groups)
    NCHMAX = max(len(ck) for g in groups for ck in g["tile_chunks"].values())
    IDXWMAX = max(sum(g["blen"]) for g in groups) // 16

    # quarter boundaries in groups: AG_q can fire once group `qgrp[q]` done
    qgrp = [min((QS[q] + TQ[q] + GT - 1) // GT - 1, NG - 1) for q in range(NB)]

    with tile.TileContext(nc) as tc:
        with (
            tc.tile_pool(name="dram", bufs=1, space="DRAM") as dpool,
            tc.tile_pool(name="const", bufs=1) as cp,
            tc.tile_pool(name="resid", bufs=1) as rp,
        ):
            p_chunk = dpool.tile([MP, HID], GD)
            h_chunk = dpool.tile([MP, HID], GD)
            # +P pad rows so a 512B (2-row) descriptor at the last real row
            # stays in bounds; pad content is garbage and never consumed
            PADR = P if WIDE_GATHER else 0
            p_bank = [
                dpool.tile([NCORES * TQ[q] * P + PADR, HID], GD,
                           addr_space="Shared", name=f"p_bank{q}")
                for q in range(NB)
            ]
            h_bank = [
                dpool.tile([NCORES * TQ[q] * P + PADR, HID], GD,
                           addr_space="Shared", name=f"h_bank{q}")
                for q in range(NB)
            ]

            w1a = cp.tile([P, HID], BF)
            w1b = cp.tile([P, HID], BF)
            w2s = cp.tile([HID, OUTD], BF)
            b1r = cp.tile([1, HID], BF)
            b2r = cp.tile([1, OUTD], BF)
            sqd = cp.tile([1, NT, P], BF)
            iota = cp.tile([P, 1, P], BF)
            dinv = cp.tile([P, NT], F32)
            dinv2 = cp.tile([P, NT], F32)
            ident = cp.tile([P, P], BF)
            nc.sync.dma_start(w1a[:], w1_d[0:P, :])
            nc.sync.dma_start(w1b[:], w1_d[P:IND, :])
            nc.sync.dma_start(w2s[:], w2_d[:])
            nc.sync.dma_start(b1r[:], b1_d[:])
            nc.sync.dma_start(b2r[:], b2_d[:])
            nc.sync.dma_start(sqd[:], sqd_d[:])
            nc.sync.dma_start(iota[:], iota_d[:])
            nc.sync.dma_start(dinv[:], dinv_d[:])
            nc.sync.dma_start(dinv2[:], dinv2_d[:])
            make_identity(nc, ident[:])

            # resident per-core feature copies (also DMA'd to DRAM for AG)
            p_sb = rp.tile([P, NT, HID], BF)
            h_sb = rp.tile([P, NT, HID], BF)
            # last tile only has LAST_ROWS valid rows; zero the tail lanes
            nc.vector.memset(p_sb[:, NT - 1, :], 0.0)
            nc.vector.memset(h_sb[:, NT - 1, :], 0.0)
            if GATHER_FP8:
                # fp8 copies feeding the AG chunks (self-loop stays bf16)
                p8_sb = rp.tile([P, NT, HID], GD)
                h8_sb = rp.tile([P, NT, HID], GD)
                nc.vector.memset(p8_sb[:, NT - 1, :], 0.0)
                nc.vector.memset(h8_sb[:, NT - 1, :], 0.0)
            else:
                p8_sb, h8_sb = p_sb, h_sb

            def emit_ag(q, chunk, banks):
                if "ag" in ablate:
                    return
                q0 = QS[q] * P
                q1 = (QS[q] + TQ[q]) * P
                nc.gpsimd.collective_compute(
                    "AllGather", mybir.AluOpType.bypass,
                    replica_groups=[list(range(NCORES))],
                    ins=[chunk[q0:q1, :].opt()],
                    outs=[banks[q][0 : NCORES * TQ[q] * P, :].opt()],
                )

            # ---------------- stage A: P' = (dinv.x) @ W1 ----------------
            NGA = (NT + GA - 1) // GA
            qgrpA = [min((QS[q] + TQ[q] + GA - 1) // GA - 1, NGA - 1)
                     for q in range(NB)]
            for _rep in range(REPEAT):
              with (
                  tc.tile_pool(name="sa", bufs=2) as sa,
                  tc.tile_pool(name="psA", bufs=2, space="PSUM") as psA,
              ):
                  for g_i in range(NGA):
                      tiles = list(range(g_i * GA, min((g_i + 1) * GA, NT)))
                      c0 = tiles[0] * P
                      c1 = min(c0 + GA * P, M)
                      cols = c1 - c0
                      xa = sa.tile([P, GA * P], BF, tag="xa")
                      xb = sa.tile([P, GA * P], BF, tag="xb")
                      nc.sync.dma_start(xa[:, :cols], xT_d[0:P, c0:c1])
                      nc.sync.dma_start(xb[:, :cols], xT_d[P:IND, c0:c1])
                      for j, t in enumerate(tiles):
                          rows = P if t < NT - 1 else LAST_ROWS
                          ps = psA.tile([P, HID], mybir.dt.float32, tag="psA")
                          nc.tensor.matmul(
                              ps[:rows, :], lhsT=xa[:, j * P : j * P + rows],
                              rhs=w1a[:], start=True, stop=False,
                          )
                          nc.tensor.matmul(
                              ps[:rows, :], lhsT=xb[:, j * P : j * P + rows],
                              rhs=w1b[:], start=False, stop=True,
                          )
                          nc.scalar.activation(
                              p_sb[:rows, t, :], ps[:rows, :], ACT_COPY, bias=0.0,
                          )
                          if GATHER_FP8:
                              nc.scalar.activation(
                                  p8_sb[:rows, t, :], ps[:rows, :], ACT_COPY,
                                  bias=0.0,
                              )
                      # always full tiles into the padded chunk
                      nc.sync.dma_start(
                          p_chunk[c0 : c0 + len(tiles) * P, :].rearrange(
                              "(a p) f -> p a f", p=P
                          ),
                          p8_sb[:, tiles[0] : tiles[0] + len(tiles), :],
                      )
                      for q in range(NB):
                          if qgrpA[q] == g_i:
                              emit_ag(q, p_chunk, p_bank)

              # ------------- aggregation layers -------------
              with (
                  tc.tile_pool(name="agg", bufs=AGG_BUFS) as ag,
                  tc.tile_pool(name="mskp", bufs=2) as mskp,
                  tc.tile_pool(name="oslabp", bufs=2) as oslabp,
                  tc.tile_pool(name="ttp", bufs=3) as ttp,
                  tc.tile_pool(name="psAg", bufs=GT, space="PSUM") as psAg,
                  tc.tile_pool(name="psF", bufs=2, space="PSUM") as psF,
              ):
                  for layer in (0, 1):
                      banks = p_bank if layer == 0 else h_bank
                      if OWN_BANK:
                          banks = banks + [p_chunk if layer == 0 else h_chunk]
                      own_sb = p_sb if layer == 0 else h_sb
                      for g_i, g in enumerate(groups):
                          W16 = sum(g["blen"]) // 16
                          io = g["idx_off"] // 16
                          idxs = ag.tile([P, IDXWMAX], I16, tag="idxs")
                          nc.sync.dma_start(idxs[:, :W16], idx_d[:, io : io + W16])
                          dstl = ag.tile([P, NDMAX], BF, tag="dstl")
                          nc.sync.dma_start(
                              dstl[:, : g["nd"]],
                              dst_d[:, g["dst_off"] : g["dst_off"] + g["nd"]],
                          )
                          GW = (PAR * HID if GATHER_FP8
                                else (2 * HID if WIDE_GATHER else HID))
                          gbuf = ag.tile([P, CHMAX, GW], GD, tag="gbuf")
                          boff = 0
                          for b in range(NBK):
                              L = g["blen"][b]
                              if L == 0 or "gather" in ablate:
                                  if L and "gather" in ablate and b == 0:
                                      nc.vector.memset(gbuf[:, :, 0:2], 0.0)
                                  continue
                              nch_b = (L + P - 1) // P
                              if WIDE_GATHER:
                                  # 512B window starting at each row: overlapping
                                  # strided view, elem_step = one row
                                  bap = banks[b][:]
                                  src = type(bap)(
                                      tensor=bap.tensor,
                                      offset=bap.offset,
                                      ap=[[HID, NCORES * TQ[b] * P], [1, GW]],
                                  )
                                  nc.gpsimd.dma_gather(
                                      gbuf[:, g["gb_choff"][b] : g["gb_choff"][b] + nch_b, :],
                                      src,
                                      idxs[:, boff : boff + L // 16],
                                      L, L, GW, elem_step=HID,
                                      queue_num=b % NQUEUES,
                                      single_packet=SINGLE_PACKET,
                                  )
                              else:
                                  if GATHER_FP8:
                                      # idx addresses an aligned PAIR of fp8
                                      # rows: 256B elems satisfy the ucode
                                      # granularity; masks select the half
                                      src_ap = banks[b][:].rearrange(
                                          "(a two) f -> a (two f)", two=2
                                      )
                                      esz = PAR * HID
                                  else:
                                      src_ap = banks[b][:]
                                      esz = HID
                                  nc.gpsimd.dma_gather(
                                      gbuf[:, g["gb_choff"][b] : g["gb_choff"][b] + nch_b, :],
                                      src_ap,
                                      idxs[:, boff : boff + L // 16],
                                      L, L, esz, queue_num=b % NQUEUES,
                                      single_packet=SINGLE_PACKET,
                                  )
                              boff += L // 16
                          oslab = (
                              None
                              if layer == 0
                              else oslabp.tile([P, GT, OUTD], BF if OUT_BF16 else mybir.dt.float32,
                                           tag="oslab")
                          )
                          # one mask slab per group: all tiles' one-hot columns
                          nd = g["nd"]
                          msk = mskp.tile([P, NDMAX, P], BF, tag="msk")
                          if "st3" in ablate:
                              nc.vector.memset(msk[:, :2, 0:2], 0.0)
                          else:
                              nc.vector.tensor_tensor(
                                  out=msk[:, :nd, :],
                                  in0=iota[:].to_broadcast([P, nd, P]),
                                  in1=dstl[:, :nd]
                                  .rearrange("p (a b) -> p a b", b=1)
                                  .to_broadcast([P, nd, P]),
                                  op=mybir.AluOpType.is_equal,
                              )
                          # pass -1: self-loop (+bias) matmuls
                          tiles = g["tiles"]
                          state = {}
                          for j, t in enumerate(tiles):
                              chunks = g["tile_chunks"][t]
                              if "mm" in ablate:
                                  chunks = chunks[:1]
                              d0 = g["tile_chunks"][t][0][2] - g["dst_off"] if chunks else 0
                              ps = psAg.tile([P, P], mybir.dt.float32, tag="psAg",
                                             padded_shape=[P, 512])
                              l0bias = layer == 0 and b1_nz
                              if layer == 0:
                                  nc.tensor.matmul(
                                      ps[:], lhsT=ident[:], rhs=own_sb[:, t, :],
                                      start=True, stop=(not chunks and not l0bias),
                                  )
                              else:
                                  nc.tensor.matmul(
                                      ps[:], lhsT=own_sb[:, t, :], rhs=ident[:],
                                      start=True, stop=(not chunks),
                                  )
                              if l0bias:
                                  nc.tensor.matmul(
                                      ps[:], lhsT=sqd[0:1, t, :], rhs=b1r[:],
                                      start=False, stop=(not chunks),
                                  )
                              state[t] = (ps, d0, chunks, len(chunks))
                          # bank passes: chunk matmuls in bank arrival order
                          done_cnt = {t: 0 for t in tiles}

                          def finalize(j, t, ps):
                              rows = P if t < NT - 1 else LAST_ROWS
                              if layer == 0:
                                  nc.scalar.activation(
                                      h_sb[:rows, t, :], ps[:rows, :], ACT_RELU,
                                      scale=dinv2[:rows, t : t + 1],
                                  )
                                  if GATHER_FP8:
                                      nc.scalar.activation(
                                          h8_sb[:rows, t, :], ps[:rows, :],
                                          ACT_RELU,
                                          scale=dinv2[:rows, t : t + 1],
                                      )
                              else:
                                  tt = ttp.tile([P, P], BF, tag="tt")
                                  nc.scalar.activation(
                                      tt[:], ps[:], ACT_COPY, bias=0.0,
                                  )
                                  pf = psF.tile([P, OUTD], mybir.dt.float32, tag="psF")
                                  nc.tensor.matmul(
                                      pf[:rows, :], lhsT=tt[:, :rows], rhs=w2s[:],
                                      start=True, stop=(not b2_nz),
                                  )
                                  if b2_nz:
                                      nc.tensor.matmul(
                                          pf[:rows, :], lhsT=sqd[0:1, t, :rows],
                                          rhs=b2r[:], start=False, stop=True,
                                      )
                                  nc.scalar.activation(
                                      oslab[:rows, j, :], pf[:rows, :], ACT_COPY,
                                      scale=dinv[:rows, t : t + 1],
                                  )

                          for j, t in enumerate(tiles):
                              if not state[t][2]:
                                  finalize(j, t, state[t][0])
                          for b in range(NBK):
                              for j, t in enumerate(tiles):
                                  ps, d0, chunks, ntot = state[t]
                                  for k, (eb, chk, dc) in enumerate(chunks):
                                      if eb != b:
                                          continue
                                      done_cnt[t] += 1
                                      last = done_cnt[t] == len(chunks)
                                      col0 = dc - g["dst_off"]
                                      for pi in range(PAR):
                                          stop_now = last and pi == PAR - 1
                                          gsl = gbuf[:, chk, pi * HID : (pi + 1) * HID]
                                          if layer == 0:
                                              nc.tensor.matmul(
                                                  ps[:], lhsT=msk[:, col0 + pi, :],
                                                  rhs=gsl,
                                                  start=False, stop=stop_now,
                                              )
                                          else:
                                              nc.tensor.matmul(
                                                  ps[:], lhsT=gsl,
                                                  rhs=msk[:, col0 + pi, :],
                                                  start=False, stop=stop_now,
                                              )
                                      if last:
                                          finalize(j, t, ps)
                          # write the group's output rows
                          if layer == 0:
                              c0 = g["tiles"][0] * P
                              nc.sync.dma_start(
                                  h_chunk[c0 : c0 + len(g["tiles"]) * P, :].rearrange(
                                      "(a p) f -> p a f", p=P
                                  ),
                                  h8_sb[:, g["tiles"][0] : g["tiles"][0] + len(g["tiles"]), :],
                              )
                              for q in range(NB):
                                  if qgrp[q] == g_i:
                                      emit_ag(q, h_chunk, h_bank)
                          else:
                              c0 = g["tiles"][0] * P
                              c1 = min(c0 + GT * P, M)
                              if c1 - c0 == len(g["tiles"]) * P:
                                  nc.sync.dma_start(
                                      out_d[c0:c1, :].rearrange("(a p) f -> p a f", p=P),
                                      oslab[:, : len(g["tiles"]), :],
                                  )
                              else:
                                  for j, t in enumerate(g["tiles"]):
                                      rows = P if t < NT - 1 else LAST_ROWS
                                      r0 = c0 + j * P
                                      nc.sync.dma_start(
                                          out_d[r0 : r0 + rows, :], oslab[:rows, j, :]
                                      )
    nc.compile()
    # Align each gather's SWDGE queue with its round-robin DMASW semaphore
    # lane (lane i -> queue i%NQUEUES). The tile scheduler may reorder
    # gathers, and a lane shared by two queues can release a consumer early
    # (queue FIFOs are unordered relative to each other). Post-patch the
    # queue_num so lane and queue FIFO order always agree.
    from concourse.bass_isa import AnyDMAInstruction
    lane = 0
    for bb in nc.m.functions[0].blocks:
        for inst in bb.instructions:
            if inst.engine == mybir.EngineType.Pool and isinstance(
                inst, AnyDMAInstruction
            ):
                if hasattr(inst, "queue_num"):
                    inst.queue_num = lane % NQUEUES
                lane = (lane + 1) % 8
    return nc


_CACHE = {}


def _get_compiled(x, edge_index, W1, b1, W2, b2):
    in_maps, U, groups, TOTIDX, TOTCH, b1_nz, b2_nz = _host_prep(
        x, edge_index, W1, b1, W2, b2
    )
    key = (TOTIDX, TOTCH, ABLATE, GT, AGG_BUFS, REPEAT, b1_nz, b2_nz,
           SINGLE_PACKET, NQUEUES, PAD_IDX, OUT_BF16, ALIGN, GATHER_FP8,
           WIDE_GATHER, OWN_BANK,
           tuple(int(v) for v in np.asarray(U).reshape(-1)[:64]))
    if key not in _CACHE:
        _CACHE[key] = _build_nc(groups, TOTIDX, TOTCH, b1_nz, b2_nz)
    return _CACHE[key], in_maps


def kernel(x, edge_index, W1, b1, W2, b2):
    from concourse.bass_utils import run_bass_kernel_spmd

    nc, in_maps = _get_compiled(x, edge_index, W1, b1, W2, b2)
    res = run_bass_kernel_spmd(nc, in_maps, core_ids=list(range(NCORES)))
    out = np.concatenate([res.results[c]["out"] for c in range(NCORES)], axis=0)
    return np.ascontiguousarray(out.astype(np.float32))

